# revision 45
# baseline (speedup 1.0000x reference)
"""Trainium2 Bass kernel for sparse-attention 3D-ViT (nn_BaseModel_44341242364529).

Strategy: shard the sequence axis L across 8 cores (512 patch rows each; the
BOS/EOS rows are replicated on every core as local tile 4). Per layer each
core computes its local q/k/v in bf16, AllGathers one fused bf16 k+v buffer
(DRAM collective), pulls a 1536-row causal band window of keys via
dynamic-offset DMAs (ds(pid,3) on the gathered chunk axis), and runs
band-dense attention: S^T blocked [128 keys, 8*128 queries] in PSUM (single
bf16 matmul pass per block) -> exp on ACT (scale folded) -> multiplicative
bf16 mask resident in SBUF (encodes geo-prior + validity + causal masking,
M = exp(bias)) on DVE -> P^T used as matmul stationary for AV with a
ones-column appended to V giving the softmax denominator for free.

All matmul operands are bf16 (PSUM accumulation stays fp32); the residual
stream x stays fp32 in SBUF.
"""

import numpy as np
import ml_dtypes

BF16 = ml_dtypes.bfloat16

# model dims (hardcoded per spec)
IMG, PATCH, D, H, NLAYERS, DFF = 64, 4, 256, 4, 2, 1024
GT = IMG // PATCH          # 16
N = GT * GT * GT           # 4096
L = N + 2                  # 4098
DH = D // H                # 64
PVOL = PATCH ** 3          # 64
NCORES = 8
LC = 512                   # real patch rows per core
LLOC = 640                 # padded local rows (5 tiles of 128)
NT = 5                     # local row tiles
SCALE = 1.0 / np.sqrt(DH)  # 0.125
NEG = -1e30

# per query tile t (0..3): window key-tiles [t, t+3..t+8] + local tile4 (BOS)
def _kts_for_tile(t):
    if t < 4:
        return [("win", t), ("win", t + 3), ("win", t + 4), ("win", t + 5),
                ("win", t + 6), ("win", t + 7), ("win", t + 8), ("loc4", 0)]
    # tile 4 = BOS/EOS rows: local tile4 keys + gathered global tiles 29, 31
    return [("loc4", 0), ("x", 0), ("x", 1)]


_prog_cache = {}


def _build_program(zero_flags):
    import concourse.bass as bass
    import concourse.bacc as bacc
    import concourse.tile as tile
    from concourse import mybir

    f32 = mybir.dt.float32
    bf16 = mybir.dt.bfloat16
    AF = mybir.ActivationFunctionType
    nc = bacc.Bacc("TRN2", target_bir_lowering=False, debug=False,
                   num_devices=NCORES)

    # ---------------- I/O declarations ----------------
    def din(name, shape, dt=bf16):
        return nc.declare_dram_parameter(name, list(shape), dt, isOutput=False)

    imgT_d = din("imgT", [PVOL, LLOC])
    emb_d = din("emb", [LLOC, D], f32)
    ident_d = din("ident", [128, 128])
    wq_d = din("wq", [NLAYERS, D, D])
    wk_d = din("wk", [NLAYERS, D, D])
    wv_d = din("wv", [NLAYERS, D, D])
    wo_d = din("wo", [NLAYERS, D, D])
    w1_d = din("w1", [NLAYERS, D, DFF])
    w2_d = din("w2", [NLAYERS, DFF, D])
    pw_d = din("patch_w", [PVOL, D])
    maskA_d = din("maskA", [4, H, 128, 8 * 128])     # query tiles 0..3
    maskB_d = din("maskB", [H, 128, 3 * 128])        # query tile 4
    out_d = nc.declare_dram_parameter("out", [LLOC, D], f32, isOutput=True)

    # internal DRAM for the fused k+v collective (bf16)
    # layout cols: [0:1024] = kT (dt,512), [1024:2048] = v (t,h,dh)
    kv_cc = nc.dram_tensor("kv_cc", [128, 2048], bf16)
    kv_gat = nc.dram_tensor("kv_gat", [NCORES + 2, 128, 2048], bf16,
                            addr_space="Shared")

    from contextlib import ExitStack
    with tile.TileContext(nc) as tc, ExitStack() as ctx:
        sing = ctx.enter_context(tc.tile_pool(name="sing", bufs=1))
        wk_pool = ctx.enter_context(tc.tile_pool(name="wrk", bufs=1))
        wk2_pool = ctx.enter_context(tc.tile_pool(name="wrk2", bufs=2))
        p_pool = ctx.enter_context(tc.tile_pool(name="pp", bufs=3))
        ps_big = ctx.enter_context(tc.tile_pool(name="psb", bufs=3, space="PSUM"))
        ps_sm = ctx.enter_context(tc.tile_pool(name="pss", bufs=1, space="PSUM"))
        ps_tr = ctx.enter_context(tc.tile_pool(name="pst", bufs=1, space="PSUM"))

        sync = nc.sync
        pid = sync.partition_id()

        # ---------------- load constants/weights ----------------
        ident = sing.tile([128, 128], bf16, tag="ident")
        sync.dma_start(out=ident[:], in_=ident_d[:, :])
        imgT = sing.tile([PVOL, LLOC], bf16, tag="imgT")
        sync.dma_start(out=imgT[:], in_=imgT_d[:, :])
        emb = sing.tile([128, NT, D], f32, tag="emb")
        sync.dma_start(out=emb[:], in_=emb_d.rearrange("(t p) d -> p t d", p=128))
        pw = sing.tile([PVOL, D], bf16, tag="pw")
        sync.dma_start(out=pw[:], in_=pw_d[:, :])

        W = {}
        for nm, dt_, kd in (("wq", wq_d, 2), ("wk", wk_d, 2), ("wv", wv_d, 2),
                            ("wo", wo_d, 2), ("w1", w1_d, 2), ("w2", w2_d, 8)):
            nout = dt_.shape[2]
            for l in range(NLAYERS):
                t_ = sing.tile([128, kd, nout], bf16, tag=f"{nm}{l}")
                sync.dma_start(out=t_[:], in_=dt_[l].rearrange("(k p) n -> p k n", p=128))
                W[(nm, l)] = t_

        # resident multiplicative masks (bf16)
        mA = sing.tile([128, 4, H, 8 * 128], bf16, tag="mA")
        sync.dma_start(out=mA[:], in_=maskA_d.rearrange("t h p x -> p t h x"))
        mB = sing.tile([128, H, 3 * 128], bf16, tag="mB")
        sync.dma_start(out=mB[:], in_=maskB_d.rearrange("h p x -> p h x"))

        # zero the 2 pad chunks of the gathered buffer (avoid NaN garbage)
        zt = sing.tile([128, 2048], bf16, tag="zero")
        nc.vector.memset(zt[:], 0.0)
        for ch in range(2):
            sync.dma_start(out=kv_gat[ch], in_=zt[:])

        eps_sb = sing.tile([128, 1], f32, tag="eps")
        nc.vector.memset(eps_sb[:], 1e-5)

        # persistent activations
        x_sb = wk_pool.tile([128, NT, D], f32, tag="x")
        # window k^T, one tile per feature-half so the S matmuls of heads 0/1
        # only gate on the dt=0 window DMA: [p(dh), chunk, 512]
        kT_win0 = wk_pool.tile([128, 3, 512], bf16, tag="kwin0")
        kT_win1 = wk_pool.tile([128, 3, 512], bf16, tag="kwin1")
        kT_wins = (kT_win0, kT_win1)
        # window v: [p(key row), chunk, lt, head, dh+1] (ones col written once)
        v_win = wk_pool.tile([128, 3, 4, H, DH + 1], bf16, tag="vwin")
        kT_x = wk_pool.tile([128, 2, 2, 128], bf16, tag="kx")
        v_x = wk_pool.tile([128, 2, H, DH + 1], bf16, tag="vx")
        v_ext = wk_pool.tile([128, NT, H, DH + 1], bf16, tag="vext")
        nc.vector.memset(v_win[:, :, :, :, DH:DH + 1], 1.0)
        nc.vector.memset(v_x[:, :, :, DH:DH + 1], 1.0)
        nc.vector.memset(v_ext[:, :, :, DH:DH + 1], 1.0)

        # ---------------- patch embed ----------------
        for lt in range(NT):
            ps = ps_sm.tile([128, 260], f32, tag="sm")
            nc.tensor.matmul(ps[:, 0:D], lhsT=imgT[:, lt * 128:(lt + 1) * 128],
                             rhs=pw[:], start=True, stop=True)
            nc.vector.tensor_add(x_sb[:, lt, :], ps[:, 0:D], emb[:, lt, :])

        # ---------------- helpers ----------------
        def layer_norm(src, dst, sname):
            """row-wise LN over D; scale/bias skipped when trivially 1/0."""
            for lt in range(NT):
                stats = wk2_pool.tile([128, 6], f32, tag="bns")
                mv = wk2_pool.tile([128, 2], f32, tag="bnm")
                nc.vector.bn_stats(out=stats[:], in_=src[:, lt, :])
                nc.vector.bn_aggr(out=mv[:], in_=stats[:])
                rstd = wk2_pool.tile([128, 1], f32, tag="rstd")
                nc.scalar.activation(out=rstd[:], in_=mv[:, 1:2], func=AF.Sqrt,
                                     bias=eps_sb[:], scale=1.0)
                nc.vector.reciprocal(out=rstd[:], in_=rstd[:])
                nc.vector.tensor_scalar(out=dst[:, lt, :], in0=src[:, lt, :],
                                        scalar1=mv[:, 0:1], scalar2=rstd[:],
                                        op0=mybir.AluOpType.subtract,
                                        op1=mybir.AluOpType.mult)
                if not zero_flags[sname]:
                    sc = W[("lns", sname)]
                    nc.vector.tensor_mul(dst[:, lt, :], dst[:, lt, :], sc[:, 0, :])
                    nc.vector.tensor_add(dst[:, lt, :], dst[:, lt, :], sc[:, 1, :])

        def transpose_tiles(src_sb, lt, dst_sb):
            """src [128l, 256] tile lt -> dst^T [128, 2, *] cols lt*128.."""
            for dt_ in range(2):
                pt = ps_tr.tile([128, 128], bf16, tag="tr")
                nc.tensor.transpose(pt[:], src_sb[:, lt, dt_ * 128:(dt_ + 1) * 128],
                                    ident[:])
                nc.vector.tensor_copy(out=dst_sb[:, dt_, lt * 128:(lt + 1) * 128],
                                      in_=pt[:])

        # LN scale/bias tiles if needed
        for nm in ("ln1_0", "ln2_0", "ln1_1", "ln2_1", "lnf"):
            if not zero_flags[nm]:
                t_ = sing.tile([128, 2, D], f32, tag=f"lns_{nm}")
                W[("lns", nm)] = t_
                dd = nc.declare_dram_parameter(f"lnsb_{nm}", [2, D], f32, isOutput=False)
                sync.dma_start(out=t_[:], in_=dd.to_broadcast([128, 2, D]))

        h_sb = wk_pool.tile([128, NT, D], bf16, tag="h")
        hf_sb = wk_pool.tile([128, NT, D], f32, tag="hf")
        hT = wk_pool.tile([128, 2, LLOC], bf16, tag="hT")
        qT = wk_pool.tile([128, 2, LLOC], bf16, tag="qT")
        kT = wk_pool.tile([128, 2, LLOC], bf16, tag="kT")
        yT_sb = wk_pool.tile([128, 8, LLOC], bf16, tag="yT")

        # ---------------- layers ----------------
        for l in range(NLAYERS):
            layer_norm(x_sb, h_sb, f"ln1_{l}")
            for lt in range(NT):
                transpose_tiles(h_sb, lt, hT)

            # k^T feature-major [128, 2, 640] (first: feeds the collective)
            def projT(nm, dstT):
                wsb = W[(nm, l)]
                for j in range(2):
                    ps = ps_big.tile([128, 1024], f32, tag="big")
                    for i in range(2):
                        nc.tensor.matmul(ps[:, 0:512],
                                         lhsT=wsb[:, i, j * 128:(j + 1) * 128],
                                         rhs=hT[:, i, 0:512],
                                         start=(i == 0), stop=(i == 1))
                        nc.tensor.matmul(ps[:, 512:640],
                                         lhsT=wsb[:, i, j * 128:(j + 1) * 128],
                                         rhs=hT[:, i, 512:640],
                                         start=(i == 0), stop=(i == 1))
                    nc.vector.tensor_copy(out=dstT[:, j, :], in_=ps[:, 0:LLOC])

            projT("wk", kT)

            # v row-major [128, 5, H, 65] (ones column persistent)
            wsb = W[("wv", l)]
            for lt in range(NT):
                ps = ps_sm.tile([128, 260], f32, tag="sm")
                for i in range(2):
                    nc.tensor.matmul(ps[:, 0:D],
                                     lhsT=hT[:, i, lt * 128:(lt + 1) * 128],
                                     rhs=wsb[:, i, :], start=(i == 0), stop=(i == 1))
                nc.vector.tensor_copy(
                    out=v_ext[:, lt, :, 0:DH],
                    in_=ps[:, 0:D].rearrange("p (h x) -> p h x", h=H))

            # ---- fused k+v AllGather (bf16) ----
            sync.dma_start(out=kv_cc[:, 0:1024].rearrange("p (a x) -> p a x", a=2),
                           in_=kT[:, :, 0:LC])
            sync.dma_start(out=kv_cc[:, 1024:2048].rearrange(
                               "p (t h x) -> p t h x", t=4, h=H),
                           in_=v_ext[:, 0:4, :, 0:DH])
            nc.gpsimd.collective_compute(
                "AllGather", mybir.AluOpType.bypass,
                replica_groups=[list(range(NCORES))],
                ins=[kv_cc[:, :].opt()],
                outs=[kv_gat[2:NCORES + 2].opt()])

            # q^T overlaps the collective (local only)
            projT("wq", qT)

            # ---- window DMAs (dynamic chunk offset = pid) ----
            for dt_ in range(2):
                sync.dma_start(
                    out=kT_wins[dt_][:, :, :],
                    in_=kv_gat[bass.ds(pid, 3), :, dt_ * 512:(dt_ + 1) * 512]
                        .rearrange("c p x -> p c x"))
            for c_ in range(3):
                sync.dma_start(
                    out=v_win[:, c_, :, :, 0:DH],
                    in_=kv_gat[bass.ds(pid + c_, 1), :, 1024:2048]
                        .rearrange("c p (t h x) -> p (c t) h x", t=4, h=H))
            # tile-4 extra keys: global patch tiles 29 and 31 (chunk 7 -> gat 9)
            for xi, base in ((0, 128), (1, 384)):
                sync.dma_start(
                    out=kT_x[:, :, xi, :],
                    in_=kv_gat[9, :, 0:1024].rearrange("p (a x) -> p a x", x=512)
                        [:, :, base:base + 128])
            for xi, tx in ((0, 1), (1, 3)):
                sync.dma_start(
                    out=v_x[:, xi, :, 0:DH],
                    in_=kv_gat[9, :, 1024 + tx * 256:1024 + (tx + 1) * 256]
                        .rearrange("p (h x) -> p h x", h=H))

            # ---- attention per query tile / head ----
            for t in range(NT):
                kts = _kts_for_tile(t)
                nkt = len(kts)
                ao_ps = ps_sm.tile([128, 260], f32, tag="sm")
                for hh in range(H):
                    pb, dt_ = (hh % 2) * 64, hh // 2
                    st = ps_big.tile([128, 1024], f32, tag="big")
                    for ki, (kind, w) in enumerate(kts):
                        if kind == "win":
                            lhsT = kT_wins[dt_][pb:pb + 64, w // 4,
                                               (w % 4) * 128:(w % 4 + 1) * 128]
                        elif kind == "loc4":
                            lhsT = kT[pb:pb + 64, dt_, 512:640]
                        else:
                            lhsT = kT_x[pb:pb + 64, dt_, w, :]
                        nc.tensor.matmul(st[:, ki * 128:(ki + 1) * 128], lhsT=lhsT,
                                         rhs=qT[pb:pb + 64, dt_, t * 128:(t + 1) * 128],
                                         start=True, stop=True)
                    # exp (scale folded), then multiplicative mask (bf16, DVE 2x)
                    pe = p_pool.tile([128, 1024], bf16, tag="pe")
                    nc.scalar.activation(out=pe[:, 0:nkt * 128],
                                         in_=st[:, 0:nkt * 128],
                                         func=AF.Exp, scale=float(SCALE))
                    pt_sb = p_pool.tile([128, 1024], bf16, tag="pt")
                    if t < 4:
                        msk = mA[:, t, hh, 0:nkt * 128]
                    else:
                        msk = mB[:, hh, :]
                    nc.vector.tensor_mul(pt_sb[:, 0:nkt * 128],
                                         pe[:, 0:nkt * 128], msk)
                    for ki, (kind, w) in enumerate(kts):
                        if kind == "win":
                            rhs = v_win[:, w // 4, w % 4, hh, :]
                        elif kind == "loc4":
                            rhs = v_ext[:, 4, hh, :]
                        else:
                            rhs = v_x[:, w, hh, :]
                        nc.tensor.matmul(ao_ps[:, hh * 65:hh * 65 + 65],
                                         lhsT=pt_sb[:, ki * 128:(ki + 1) * 128],
                                         rhs=rhs, start=(ki == 0), stop=(ki == nkt - 1))
                # normalize: divide by denom col, pack to row-major [128, 256]
                rec = wk2_pool.tile([128, 4], f32, tag="rec")
                nc.vector.reciprocal(out=rec[:], in_=ao_ps[:, 64:260:65])
                ao_sb = wk2_pool.tile([128, D], bf16, tag="ao")
                for hh in range(H):
                    nc.vector.tensor_scalar(
                        out=ao_sb[:, hh * DH:(hh + 1) * DH],
                        in0=ao_ps[:, hh * 65:hh * 65 + DH],
                        scalar1=rec[:, hh:hh + 1], scalar2=None,
                        op0=mybir.AluOpType.mult)
                # wo projection + residual
                aoT = wk2_pool.tile([128, 2, 128], bf16, tag="aoT")
                for dt_ in range(2):
                    ptr = ps_tr.tile([128, 128], bf16, tag="tr")
                    nc.tensor.transpose(ptr[:], ao_sb[:, dt_ * 128:(dt_ + 1) * 128],
                                        ident[:])
                    nc.vector.tensor_copy(out=aoT[:, dt_, :], in_=ptr[:])
                xo = ps_sm.tile([128, 260], f32, tag="sm")
                wsb = W[("wo", l)]
                for i in range(2):
                    nc.tensor.matmul(xo[:, 0:D], lhsT=aoT[:, i, :], rhs=wsb[:, i, :],
                                     start=(i == 0), stop=(i == 1))
                nc.vector.tensor_add(x_sb[:, t, :], x_sb[:, t, :], xo[:, 0:D])

            # ---- FFN ----
            layer_norm(x_sb, h_sb, f"ln2_{l}")
            for lt in range(NT):
                transpose_tiles(h_sb, lt, hT)
            w1sb = W[("w1", l)]
            for fj in range(8):
                ps = ps_big.tile([128, 1024], f32, tag="big")
                for i in range(2):
                    nc.tensor.matmul(ps[:, 0:512],
                                     lhsT=w1sb[:, i, fj * 128:(fj + 1) * 128],
                                     rhs=hT[:, i, 0:512], start=(i == 0), stop=(i == 1))
                    nc.tensor.matmul(ps[:, 512:640],
                                     lhsT=w1sb[:, i, fj * 128:(fj + 1) * 128],
                                     rhs=hT[:, i, 512:640], start=(i == 0), stop=(i == 1))
                nc.scalar.activation(out=yT_sb[:, fj, :], in_=ps[:, 0:LLOC],
                                     func=AF.Gelu, scale=1.0)
            w2sb = W[("w2", l)]
            for lt in range(NT):
                ps = ps_sm.tile([128, 260], f32, tag="sm")
                for fj in range(8):
                    nc.tensor.matmul(ps[:, 0:D],
                                     lhsT=yT_sb[:, fj, lt * 128:(lt + 1) * 128],
                                     rhs=w2sb[:, fj, :], start=(fj == 0), stop=(fj == 7))
                nc.vector.tensor_add(x_sb[:, lt, :], x_sb[:, lt, :], ps[:, 0:D])

        # ---------------- final LN + output ----------------
        layer_norm(x_sb, hf_sb, "lnf")
        for lt in range(NT):
            sync.dma_start(out=out_d[lt * 128:(lt + 1) * 128, :], in_=hf_sb[:, lt, :])

    nc.finalize()
    return nc


# ======================= host side =======================

def _patchify(img):
    x = img.reshape(1, 1, GT, PATCH, GT, PATCH, GT, PATCH)
    x = np.einsum("nctphqwr->nthwpqrc", x).reshape(N, PVOL)
    return np.ascontiguousarray(x).astype(np.float32)


def _host_prep(inputs):
    idx = np.asarray(inputs["idx"])
    valid = np.asarray(inputs["valid"])
    geo = np.asarray(inputs["geo_dist"]).astype(np.float32)
    decay = np.asarray(inputs["decay"]).astype(np.float32)
    K = idx.shape[1]
    fv = valid & (idx <= np.arange(L)[:, None])
    # device computes exp(SCALE*S) * M with M = exp(geo*decay) (0 if masked)
    bias_lk = geo[None] * decay[:, None, None]  # [H, L, K]

    patches = _patchify(np.asarray(inputs["input_image"]))
    ids = np.asarray(inputs["input_ids"]).reshape(-1)
    et = np.asarray(inputs["embed_tokens"])
    pb = np.asarray(inputs["patch_b"]).astype(np.float32)
    bos_e, eos_e = et[ids[0]], et[ids[-1]]

    per_core = []
    for c in range(NCORES):
        imgT = np.zeros((PVOL, LLOC), np.float32)
        imgT[:, 0:LC] = patches[c * LC:(c + 1) * LC].T
        emb = np.zeros((LLOC, D), np.float32)
        emb[0:LC] = pb[None, :]
        emb[LC] = bos_e
        emb[LC + 1] = eos_e

        biasA = np.full((4, H, 128, 8 * 128), NEG, np.float32)
        biasB = np.full((H, 128, 3 * 128), NEG, np.float32)
        base = c * LC - 1024   # window global patch start
        for lq in range(LC):
            gq = 1 + c * LC + lq
            t, lcol = lq // 128, lq % 128
            kts = [t, t + 3, t + 4, t + 5, t + 6, t + 7, t + 8]
            for k in range(K):
                if not fv[gq, k]:
                    continue
                kr = int(idx[gq, k])
                bv = bias_lk[:, gq, k]
                if kr == 0:                      # BOS -> local tile4 slot, j=0
                    biasA[t, :, 0, 7 * 128 + lcol] = bv
                    continue
                p = kr - 1
                wp = p - base
                assert 0 <= wp < 1536, (c, gq, kr)
                w, j = wp // 128, wp % 128
                ki = kts.index(w)
                biasA[t, :, j, ki * 128 + lcol] = bv
        # padding queries (tile4 rows 2..127) attend BOS only -> finite output
        biasB[:, 0, 0 * 128 + 2:0 * 128 + 128] = 0.0
        # tile 4: BOS (l=0) and EOS (l=1) queries
        for li, gq in ((0, 0), (1, L - 1)):
            for k in range(K):
                if not fv[gq, k]:
                    continue
                kr = int(idx[gq, k])
                bv = bias_lk[:, gq, k]
                if kr == 0:
                    biasB[:, 0, 0 * 128 + li] = bv
                elif kr == L - 1:
                    biasB[:, 1, 0 * 128 + li] = bv
                else:
                    p = kr - 1
                    if 3712 <= p < 3840:
                        biasB[:, p - 3712, 1 * 128 + li] = bv
                    elif 3968 <= p < 4096:
                        biasB[:, p - 3968, 2 * 128 + li] = bv
                    else:
                        raise AssertionError((gq, kr))
        per_core.append({
            "imgT": imgT.astype(BF16), "emb": emb,
            "maskA": np.exp(biasA).astype(BF16),
            "maskB": np.exp(biasB).astype(BF16),
        })

    shared = {
        "ident": np.eye(128, dtype=np.float32).astype(BF16),
        "patch_w": np.asarray(inputs["patch_w"], np.float32).astype(BF16),
    }
    for nm in ("wq", "wk", "wv", "wo", "w1", "w2"):
        shared[nm] = np.asarray(inputs[nm], np.float32).astype(BF16)

    zero_flags = {}
    for nm, s_, b_ in (("ln1_0", inputs["ln1_s"][0], inputs["ln1_b"][0]),
                       ("ln2_0", inputs["ln2_s"][0], inputs["ln2_b"][0]),
                       ("ln1_1", inputs["ln1_s"][1], inputs["ln1_b"][1]),
                       ("ln2_1", inputs["ln2_s"][1], inputs["ln2_b"][1]),
                       ("lnf", inputs["norm_s"], inputs["norm_b"])):
        s_, b_ = np.asarray(s_), np.asarray(b_)
        triv = bool(np.all(s_ == 1.0) and np.all(b_ == 0.0))
        zero_flags[nm] = triv
        if not triv:
            shared[f"lnsb_{nm}"] = np.stack([s_, b_]).astype(np.float32)
    # residual biases: asserted zero (true for this model's setup_inputs)
    for nm in ("bo", "b1", "b2"):
        assert np.all(np.asarray(inputs[nm]) == 0.0), f"{nm} nonzero unsupported"

    return per_core, shared, zero_flags


def kernel(**inputs):
    from concourse.bass_utils import run_bass_kernel_spmd

    per_core, shared, zero_flags = _host_prep(inputs)
    key = tuple(sorted(zero_flags.items()))
    if key not in _prog_cache:
        _prog_cache[key] = _build_program(zero_flags)
    nc = _prog_cache[key]

    in_maps = []
    for c in range(NCORES):
        m = dict(shared)
        m.update(per_core[c])
        in_maps.append(m)
    import os
    trace = bool(os.environ.get("KERNEL_TRACE"))
    res = run_bass_kernel_spmd(nc, in_maps, core_ids=list(range(NCORES)),
                               trace=trace)
    global _last_exec_ns
    _last_exec_ns = res.exec_time_ns

    out = np.zeros((L, D), np.float32)
    for c in range(NCORES):
        out[1 + c * LC:1 + (c + 1) * LC] = res.results[c]["out"][0:LC]
    out[0] = res.results[0]["out"][LC]
    out[L - 1] = res.results[0]["out"][LC + 1]
    return out.reshape(1, L, D)


# revision 46
# speedup vs baseline: 1.1329x; 1.1329x over previous
"""Trainium2 Bass kernel for sparse-attention 3D-ViT (nn_BaseModel_44341242364529).

Strategy: shard the sequence axis L across 8 cores (512 patch rows each; the
BOS/EOS rows are replicated on every core as local tile 4). Per layer each
core computes its local q/k/v in bf16, AllGathers one fused bf16 k+v buffer
(DRAM collective), pulls a 1536-row causal band window of keys via
dynamic-offset DMAs (ds(pid,3) on the gathered chunk axis), and runs
band-dense attention: S^T blocked [128 keys, 8*128 queries] in PSUM (single
bf16 matmul pass per block) -> exp on ACT (scale folded) -> multiplicative
bf16 mask resident in SBUF (encodes geo-prior + validity + causal masking,
M = exp(bias)) on DVE -> P^T used as matmul stationary for AV with a
ones-column appended to V giving the softmax denominator for free.

All matmul operands are bf16 (PSUM accumulation stays fp32); the residual
stream x stays fp32 in SBUF.
"""

import numpy as np
import ml_dtypes

BF16 = ml_dtypes.bfloat16

# model dims (hardcoded per spec)
IMG, PATCH, D, H, NLAYERS, DFF = 64, 4, 256, 4, 2, 1024
GT = IMG // PATCH          # 16
N = GT * GT * GT           # 4096
L = N + 2                  # 4098
DH = D // H                # 64
PVOL = PATCH ** 3          # 64
NCORES = 8
LC = 512                   # real patch rows per core
LLOC = 640                 # padded local rows (5 tiles of 128)
NT = 5                     # local row tiles
SCALE = 1.0 / np.sqrt(DH)  # 0.125
NEG = -1e30

# per query tile t (0..3): window key-tiles [t, t+3..t+8] + local tile4 (BOS)
def _kts_for_tile(t):
    if t < 4:
        return [("win", t), ("win", t + 3), ("win", t + 4), ("win", t + 5),
                ("win", t + 6), ("win", t + 7), ("win", t + 8), ("loc4", 0)]
    # tile 4 = BOS/EOS rows: local tile4 keys + gathered global tiles 29, 31
    return [("loc4", 0), ("x", 0), ("x", 1)]


_prog_cache = {}


def _build_program(zero_flags):
    import concourse.bass as bass
    import concourse.bacc as bacc
    import concourse.tile as tile
    from concourse import mybir

    f32 = mybir.dt.float32
    bf16 = mybir.dt.bfloat16
    AF = mybir.ActivationFunctionType
    nc = bacc.Bacc("TRN2", target_bir_lowering=False, debug=False,
                   num_devices=NCORES)

    # ---------------- I/O declarations ----------------
    def din(name, shape, dt=bf16):
        return nc.declare_dram_parameter(name, list(shape), dt, isOutput=False)

    imgT_d = din("imgT", [PVOL, LLOC])
    emb_d = din("emb", [LLOC, D], f32)
    ident_d = din("ident", [128, 128])
    wq_d = din("wq", [NLAYERS, D, D])
    wk_d = din("wk", [NLAYERS, D, D])
    wv_d = din("wv", [NLAYERS, D, D])
    wo_d = din("wo", [NLAYERS, D, D])
    w1_d = din("w1", [NLAYERS, D, DFF])
    w2_d = din("w2", [NLAYERS, DFF, D])
    pw_d = din("patch_w", [PVOL, D])
    maskA_d = din("maskA", [4, H, 128, 8 * 128])     # query tiles 0..3
    maskB_d = din("maskB", [H, 128, 3 * 128])        # query tile 4
    out_d = nc.declare_dram_parameter("out", [LLOC, D], f32, isOutput=True)

    # internal DRAM for the fused k+v collective (bf16)
    # layout cols: [0:1024] = kT (dt,512), [1024:2048] = v (t,h,dh)
    kv_cc = nc.dram_tensor("kv_cc", [128, 2048], bf16)
    kv_gat = nc.dram_tensor("kv_gat", [NCORES + 2, 128, 2048], bf16,
                            addr_space="Shared")

    from contextlib import ExitStack
    with tile.TileContext(nc) as tc, ExitStack() as ctx:
        sing = ctx.enter_context(tc.tile_pool(name="sing", bufs=1))
        wk_pool = ctx.enter_context(tc.tile_pool(name="wrk", bufs=1))
        wk2_pool = ctx.enter_context(tc.tile_pool(name="wrk2", bufs=2))
        p_pool = ctx.enter_context(tc.tile_pool(name="pp", bufs=3))
        ps_big = ctx.enter_context(tc.tile_pool(name="psb", bufs=2, space="PSUM"))
        ps_sm = ctx.enter_context(tc.tile_pool(name="pss", bufs=2, space="PSUM"))
        ps_tr = ctx.enter_context(tc.tile_pool(name="pst", bufs=2, space="PSUM"))

        sync = nc.sync
        pid = sync.partition_id()

        # ---------------- load constants/weights ----------------
        ident = sing.tile([128, 128], bf16, tag="ident")
        sync.dma_start(out=ident[:], in_=ident_d[:, :])
        imgT = sing.tile([PVOL, LLOC], bf16, tag="imgT")
        sync.dma_start(out=imgT[:], in_=imgT_d[:, :])
        emb = sing.tile([128, NT, D], f32, tag="emb")
        sync.dma_start(out=emb[:], in_=emb_d.rearrange("(t p) d -> p t d", p=128))
        pw = sing.tile([PVOL, D], bf16, tag="pw")
        sync.dma_start(out=pw[:], in_=pw_d[:, :])

        W = {}
        for nm, dt_, kd in (("wq", wq_d, 2), ("wk", wk_d, 2), ("wv", wv_d, 2),
                            ("wo", wo_d, 2), ("w1", w1_d, 2), ("w2", w2_d, 8)):
            nout = dt_.shape[2]
            for l in range(NLAYERS):
                t_ = sing.tile([128, kd, nout], bf16, tag=f"{nm}{l}")
                sync.dma_start(out=t_[:], in_=dt_[l].rearrange("(k p) n -> p k n", p=128))
                W[(nm, l)] = t_

        # resident multiplicative masks (bf16)
        mA = sing.tile([128, 4, H, 8 * 128], bf16, tag="mA")
        sync.dma_start(out=mA[:], in_=maskA_d.rearrange("t h p x -> p t h x"))
        mB = sing.tile([128, H, 3 * 128], bf16, tag="mB")
        sync.dma_start(out=mB[:], in_=maskB_d.rearrange("h p x -> p h x"))

        # zero the 2 pad chunks of the gathered buffer (avoid NaN garbage)
        zt = sing.tile([128, 2048], bf16, tag="zero")
        nc.vector.memset(zt[:], 0.0)
        for ch in range(2):
            sync.dma_start(out=kv_gat[ch], in_=zt[:])

        eps_sb = sing.tile([128, 1], f32, tag="eps")
        nc.vector.memset(eps_sb[:], 1e-5)

        # persistent activations
        x_sb = wk_pool.tile([128, NT, D], f32, tag="x")
        # window k^T, one tile per feature-half so the S matmuls of heads 0/1
        # only gate on the dt=0 window DMA: [p(dh), chunk, 512]
        kT_win0 = wk_pool.tile([128, 3, 512], bf16, tag="kwin0")
        kT_win1 = wk_pool.tile([128, 3, 512], bf16, tag="kwin1")
        kT_wins = (kT_win0, kT_win1)
        # window v: [p(key row), chunk, lt, head, dh+1] (ones col written once)
        v_win = wk_pool.tile([128, 3, 4, H, DH + 1], bf16, tag="vwin")
        kT_x = wk_pool.tile([128, 2, 2, 128], bf16, tag="kx")
        v_x = wk_pool.tile([128, 2, H, DH + 1], bf16, tag="vx")
        v_ext = wk_pool.tile([128, NT, H, DH + 1], bf16, tag="vext")
        nc.vector.memset(v_win[:, :, :, :, DH:DH + 1], 1.0)
        nc.vector.memset(v_x[:, :, :, DH:DH + 1], 1.0)
        nc.vector.memset(v_ext[:, :, :, DH:DH + 1], 1.0)

        # ---------------- patch embed ----------------
        for lt in range(NT):
            ps = ps_sm.tile([128, 260], f32, tag="sm")
            nc.tensor.matmul(ps[:, 0:D], lhsT=imgT[:, lt * 128:(lt + 1) * 128],
                             rhs=pw[:], start=True, stop=True)
            nc.vector.tensor_add(x_sb[:, lt, :], ps[:, 0:D], emb[:, lt, :])

        # ---------------- helpers ----------------
        def layer_norm(src, dst, sname):
            """row-wise LN over D; scale/bias skipped when trivially 1/0."""
            for lt in range(NT):
                stats = wk2_pool.tile([128, 6], f32, tag="bns")
                mv = wk2_pool.tile([128, 2], f32, tag="bnm")
                nc.vector.bn_stats(out=stats[:], in_=src[:, lt, :])
                nc.vector.bn_aggr(out=mv[:], in_=stats[:])
                rstd = wk2_pool.tile([128, 1], f32, tag="rstd")
                nc.scalar.activation(out=rstd[:], in_=mv[:, 1:2], func=AF.Sqrt,
                                     bias=eps_sb[:], scale=1.0)
                nc.vector.reciprocal(out=rstd[:], in_=rstd[:])
                nc.vector.tensor_scalar(out=dst[:, lt, :], in0=src[:, lt, :],
                                        scalar1=mv[:, 0:1], scalar2=rstd[:],
                                        op0=mybir.AluOpType.subtract,
                                        op1=mybir.AluOpType.mult)
                if not zero_flags[sname]:
                    sc = W[("lns", sname)]
                    nc.vector.tensor_mul(dst[:, lt, :], dst[:, lt, :], sc[:, 0, :])
                    nc.vector.tensor_add(dst[:, lt, :], dst[:, lt, :], sc[:, 1, :])

        def transpose_tiles(src_sb, lt, dst_sb):
            """src [128l, 256] tile lt -> dst^T [128, 2, *] cols lt*128.."""
            for dt_ in range(2):
                pt = ps_tr.tile([128, 128], bf16, tag="tr")
                nc.tensor.transpose(pt[:], src_sb[:, lt, dt_ * 128:(dt_ + 1) * 128],
                                    ident[:])
                nc.vector.tensor_copy(out=dst_sb[:, dt_, lt * 128:(lt + 1) * 128],
                                      in_=pt[:])

        # LN scale/bias tiles if needed
        for nm in ("ln1_0", "ln2_0", "ln1_1", "ln2_1", "lnf"):
            if not zero_flags[nm]:
                t_ = sing.tile([128, 2, D], f32, tag=f"lns_{nm}")
                W[("lns", nm)] = t_
                dd = nc.declare_dram_parameter(f"lnsb_{nm}", [2, D], f32, isOutput=False)
                sync.dma_start(out=t_[:], in_=dd.to_broadcast([128, 2, D]))

        h_sb = wk_pool.tile([128, NT, D], bf16, tag="h")
        hf_sb = wk_pool.tile([128, NT, D], f32, tag="hf")
        hT = wk_pool.tile([128, 2, LLOC], bf16, tag="hT")
        qT = wk_pool.tile([128, 2, LLOC], bf16, tag="qT")
        kT = wk_pool.tile([128, 2, LLOC], bf16, tag="kT")
        yT_sb = wk_pool.tile([128, 8, LLOC], bf16, tag="yT")

        # ---------------- layers ----------------
        for l in range(NLAYERS):
            layer_norm(x_sb, h_sb, f"ln1_{l}")
            for lt in range(NT):
                transpose_tiles(h_sb, lt, hT)

            # k^T feature-major [128, 2, 640] (first: feeds the collective)
            def projT(nm, dstT):
                wsb = W[(nm, l)]
                for j in range(2):
                    ps = ps_big.tile([128, 1024], f32, tag="big")
                    for i in range(2):
                        nc.tensor.matmul(ps[:, 0:512],
                                         lhsT=wsb[:, i, j * 128:(j + 1) * 128],
                                         rhs=hT[:, i, 0:512],
                                         start=(i == 0), stop=(i == 1))
                        nc.tensor.matmul(ps[:, 512:640],
                                         lhsT=wsb[:, i, j * 128:(j + 1) * 128],
                                         rhs=hT[:, i, 512:640],
                                         start=(i == 0), stop=(i == 1))
                    nc.vector.tensor_copy(out=dstT[:, j, :], in_=ps[:, 0:LLOC])

            projT("wk", kT)

            # v row-major [128, 5, H, 65] (ones column persistent)
            wsb = W[("wv", l)]
            for lt in range(NT):
                ps = ps_sm.tile([128, 260], f32, tag="sm")
                for i in range(2):
                    nc.tensor.matmul(ps[:, 0:D],
                                     lhsT=hT[:, i, lt * 128:(lt + 1) * 128],
                                     rhs=wsb[:, i, :], start=(i == 0), stop=(i == 1))
                nc.vector.tensor_copy(
                    out=v_ext[:, lt, :, 0:DH],
                    in_=ps[:, 0:D].rearrange("p (h x) -> p h x", h=H))

            # ---- fused k+v AllGather (bf16) ----
            sync.dma_start(out=kv_cc[:, 0:1024].rearrange("p (a x) -> p a x", a=2),
                           in_=kT[:, :, 0:LC])
            sync.dma_start(out=kv_cc[:, 1024:2048].rearrange(
                               "p (t h x) -> p t h x", t=4, h=H),
                           in_=v_ext[:, 0:4, :, 0:DH])
            nc.gpsimd.collective_compute(
                "AllGather", mybir.AluOpType.bypass,
                replica_groups=[list(range(NCORES))],
                ins=[kv_cc[:, :].opt()],
                outs=[kv_gat[2:NCORES + 2].opt()])

            # q^T overlaps the collective (local only)
            projT("wq", qT)

            # ---- window DMAs (dynamic chunk offset = pid) ----
            for dt_ in range(2):
                sync.dma_start(
                    out=kT_wins[dt_][:, :, :],
                    in_=kv_gat[bass.ds(pid, 3), :, dt_ * 512:(dt_ + 1) * 512]
                        .rearrange("c p x -> p c x"))
            for c_ in range(3):
                sync.dma_start(
                    out=v_win[:, c_, :, :, 0:DH],
                    in_=kv_gat[bass.ds(pid + c_, 1), :, 1024:2048]
                        .rearrange("c p (t h x) -> p (c t) h x", t=4, h=H))
            # tile-4 extra keys: global patch tiles 29 and 31 (chunk 7 -> gat 9)
            for xi, base in ((0, 128), (1, 384)):
                sync.dma_start(
                    out=kT_x[:, :, xi, :],
                    in_=kv_gat[9, :, 0:1024].rearrange("p (a x) -> p a x", x=512)
                        [:, :, base:base + 128])
            for xi, tx in ((0, 1), (1, 3)):
                sync.dma_start(
                    out=v_x[:, xi, :, 0:DH],
                    in_=kv_gat[9, :, 1024 + tx * 256:1024 + (tx + 1) * 256]
                        .rearrange("p (h x) -> p h x", h=H))

            # ---- attention per query tile / head ----
            for t in range(NT):
                kts = _kts_for_tile(t)
                nkt = len(kts)
                ao_ps = ps_sm.tile([128, 260], f32, tag="sm")
                for hh in range(H):
                    pb, dt_ = (hh % 2) * 64, hh // 2
                    st = ps_big.tile([128, 1024], f32, tag="big")
                    for ki, (kind, w) in enumerate(kts):
                        if kind == "win":
                            lhsT = kT_wins[dt_][pb:pb + 64, w // 4,
                                               (w % 4) * 128:(w % 4 + 1) * 128]
                        elif kind == "loc4":
                            lhsT = kT[pb:pb + 64, dt_, 512:640]
                        else:
                            lhsT = kT_x[pb:pb + 64, dt_, w, :]
                        nc.tensor.matmul(st[:, ki * 128:(ki + 1) * 128], lhsT=lhsT,
                                         rhs=qT[pb:pb + 64, dt_, t * 128:(t + 1) * 128],
                                         start=True, stop=True)
                    # exp (scale folded), then multiplicative mask (bf16, DVE 2x)
                    pe = p_pool.tile([128, 1024], bf16, tag="pe")
                    nc.scalar.activation(out=pe[:, 0:nkt * 128],
                                         in_=st[:, 0:nkt * 128],
                                         func=AF.Exp, scale=float(SCALE))
                    pt_sb = p_pool.tile([128, 1024], bf16, tag="pt")
                    if t < 4:
                        msk = mA[:, t, hh, 0:nkt * 128]
                    else:
                        msk = mB[:, hh, :]
                    nc.vector.tensor_mul(pt_sb[:, 0:nkt * 128],
                                         pe[:, 0:nkt * 128], msk)
                    for ki, (kind, w) in enumerate(kts):
                        if kind == "win":
                            rhs = v_win[:, w // 4, w % 4, hh, :]
                        elif kind == "loc4":
                            rhs = v_ext[:, 4, hh, :]
                        else:
                            rhs = v_x[:, w, hh, :]
                        nc.tensor.matmul(ao_ps[:, hh * 65:hh * 65 + 65],
                                         lhsT=pt_sb[:, ki * 128:(ki + 1) * 128],
                                         rhs=rhs, start=(ki == 0), stop=(ki == nkt - 1))
                # normalize: divide by denom col, pack to row-major [128, 256]
                rec = wk2_pool.tile([128, 4], f32, tag="rec")
                nc.vector.reciprocal(out=rec[:], in_=ao_ps[:, 64:260:65])
                ao_sb = wk2_pool.tile([128, D], bf16, tag="ao")
                for hh in range(H):
                    nc.vector.tensor_scalar(
                        out=ao_sb[:, hh * DH:(hh + 1) * DH],
                        in0=ao_ps[:, hh * 65:hh * 65 + DH],
                        scalar1=rec[:, hh:hh + 1], scalar2=None,
                        op0=mybir.AluOpType.mult)
                # wo projection + residual
                aoT = wk2_pool.tile([128, 2, 128], bf16, tag="aoT")
                for dt_ in range(2):
                    ptr = ps_tr.tile([128, 128], bf16, tag="tr")
                    nc.tensor.transpose(ptr[:], ao_sb[:, dt_ * 128:(dt_ + 1) * 128],
                                        ident[:])
                    nc.vector.tensor_copy(out=aoT[:, dt_, :], in_=ptr[:])
                xo = ps_sm.tile([128, 260], f32, tag="sm")
                wsb = W[("wo", l)]
                for i in range(2):
                    nc.tensor.matmul(xo[:, 0:D], lhsT=aoT[:, i, :], rhs=wsb[:, i, :],
                                     start=(i == 0), stop=(i == 1))
                nc.vector.tensor_add(x_sb[:, t, :], x_sb[:, t, :], xo[:, 0:D])

            # ---- FFN ----
            layer_norm(x_sb, h_sb, f"ln2_{l}")
            for lt in range(NT):
                transpose_tiles(h_sb, lt, hT)
            w1sb = W[("w1", l)]
            for fj in range(8):
                ps = ps_big.tile([128, 1024], f32, tag="big")
                for i in range(2):
                    nc.tensor.matmul(ps[:, 0:512],
                                     lhsT=w1sb[:, i, fj * 128:(fj + 1) * 128],
                                     rhs=hT[:, i, 0:512], start=(i == 0), stop=(i == 1))
                    nc.tensor.matmul(ps[:, 512:640],
                                     lhsT=w1sb[:, i, fj * 128:(fj + 1) * 128],
                                     rhs=hT[:, i, 512:640], start=(i == 0), stop=(i == 1))
                nc.scalar.activation(out=yT_sb[:, fj, :], in_=ps[:, 0:LLOC],
                                     func=AF.Gelu, scale=1.0)
            w2sb = W[("w2", l)]
            for lt in range(NT):
                ps = ps_sm.tile([128, 260], f32, tag="sm")
                for fj in range(8):
                    nc.tensor.matmul(ps[:, 0:D],
                                     lhsT=yT_sb[:, fj, lt * 128:(lt + 1) * 128],
                                     rhs=w2sb[:, fj, :], start=(fj == 0), stop=(fj == 7))
                nc.vector.tensor_add(x_sb[:, lt, :], x_sb[:, lt, :], ps[:, 0:D])

        # ---------------- final LN + output ----------------
        layer_norm(x_sb, hf_sb, "lnf")
        for lt in range(NT):
            sync.dma_start(out=out_d[lt * 128:(lt + 1) * 128, :], in_=hf_sb[:, lt, :])

    nc.finalize()
    return nc


# ======================= host side =======================

def _patchify(img):
    x = img.reshape(1, 1, GT, PATCH, GT, PATCH, GT, PATCH)
    x = np.einsum("nctphqwr->nthwpqrc", x).reshape(N, PVOL)
    return np.ascontiguousarray(x).astype(np.float32)


def _host_prep(inputs):
    idx = np.asarray(inputs["idx"])
    valid = np.asarray(inputs["valid"])
    geo = np.asarray(inputs["geo_dist"]).astype(np.float32)
    decay = np.asarray(inputs["decay"]).astype(np.float32)
    K = idx.shape[1]
    fv = valid & (idx <= np.arange(L)[:, None])
    # device computes exp(SCALE*S) * M with M = exp(geo*decay) (0 if masked)
    bias_lk = geo[None] * decay[:, None, None]  # [H, L, K]

    patches = _patchify(np.asarray(inputs["input_image"]))
    ids = np.asarray(inputs["input_ids"]).reshape(-1)
    et = np.asarray(inputs["embed_tokens"])
    pb = np.asarray(inputs["patch_b"]).astype(np.float32)
    bos_e, eos_e = et[ids[0]], et[ids[-1]]

    per_core = []
    for c in range(NCORES):
        imgT = np.zeros((PVOL, LLOC), np.float32)
        imgT[:, 0:LC] = patches[c * LC:(c + 1) * LC].T
        emb = np.zeros((LLOC, D), np.float32)
        emb[0:LC] = pb[None, :]
        emb[LC] = bos_e
        emb[LC + 1] = eos_e

        biasA = np.full((4, H, 128, 8 * 128), NEG, np.float32)
        biasB = np.full((H, 128, 3 * 128), NEG, np.float32)
        base = c * LC - 1024   # window global patch start
        for lq in range(LC):
            gq = 1 + c * LC + lq
            t, lcol = lq // 128, lq % 128
            kts = [t, t + 3, t + 4, t + 5, t + 6, t + 7, t + 8]
            for k in range(K):
                if not fv[gq, k]:
                    continue
                kr = int(idx[gq, k])
                bv = bias_lk[:, gq, k]
                if kr == 0:                      # BOS -> local tile4 slot, j=0
                    biasA[t, :, 0, 7 * 128 + lcol] = bv
                    continue
                p = kr - 1
                wp = p - base
                assert 0 <= wp < 1536, (c, gq, kr)
                w, j = wp // 128, wp % 128
                ki = kts.index(w)
                biasA[t, :, j, ki * 128 + lcol] = bv
        # padding queries (tile4 rows 2..127) attend BOS only -> finite output
        biasB[:, 0, 0 * 128 + 2:0 * 128 + 128] = 0.0
        # tile 4: BOS (l=0) and EOS (l=1) queries
        for li, gq in ((0, 0), (1, L - 1)):
            for k in range(K):
                if not fv[gq, k]:
                    continue
                kr = int(idx[gq, k])
                bv = bias_lk[:, gq, k]
                if kr == 0:
                    biasB[:, 0, 0 * 128 + li] = bv
                elif kr == L - 1:
                    biasB[:, 1, 0 * 128 + li] = bv
                else:
                    p = kr - 1
                    if 3712 <= p < 3840:
                        biasB[:, p - 3712, 1 * 128 + li] = bv
                    elif 3968 <= p < 4096:
                        biasB[:, p - 3968, 2 * 128 + li] = bv
                    else:
                        raise AssertionError((gq, kr))
        per_core.append({
            "imgT": imgT.astype(BF16), "emb": emb,
            "maskA": np.exp(biasA).astype(BF16),
            "maskB": np.exp(biasB).astype(BF16),
        })

    shared = {
        "ident": np.eye(128, dtype=np.float32).astype(BF16),
        "patch_w": np.asarray(inputs["patch_w"], np.float32).astype(BF16),
    }
    for nm in ("wq", "wk", "wv", "wo", "w1", "w2"):
        shared[nm] = np.asarray(inputs[nm], np.float32).astype(BF16)

    zero_flags = {}
    for nm, s_, b_ in (("ln1_0", inputs["ln1_s"][0], inputs["ln1_b"][0]),
                       ("ln2_0", inputs["ln2_s"][0], inputs["ln2_b"][0]),
                       ("ln1_1", inputs["ln1_s"][1], inputs["ln1_b"][1]),
                       ("ln2_1", inputs["ln2_s"][1], inputs["ln2_b"][1]),
                       ("lnf", inputs["norm_s"], inputs["norm_b"])):
        s_, b_ = np.asarray(s_), np.asarray(b_)
        triv = bool(np.all(s_ == 1.0) and np.all(b_ == 0.0))
        zero_flags[nm] = triv
        if not triv:
            shared[f"lnsb_{nm}"] = np.stack([s_, b_]).astype(np.float32)
    # residual biases: asserted zero (true for this model's setup_inputs)
    for nm in ("bo", "b1", "b2"):
        assert np.all(np.asarray(inputs[nm]) == 0.0), f"{nm} nonzero unsupported"

    return per_core, shared, zero_flags


def kernel(**inputs):
    from concourse.bass_utils import run_bass_kernel_spmd

    per_core, shared, zero_flags = _host_prep(inputs)
    key = tuple(sorted(zero_flags.items()))
    if key not in _prog_cache:
        _prog_cache[key] = _build_program(zero_flags)
    nc = _prog_cache[key]

    in_maps = []
    for c in range(NCORES):
        m = dict(shared)
        m.update(per_core[c])
        in_maps.append(m)
    import os
    trace = bool(os.environ.get("KERNEL_TRACE"))
    res = run_bass_kernel_spmd(nc, in_maps, core_ids=list(range(NCORES)),
                               trace=trace)
    global _last_exec_ns
    _last_exec_ns = res.exec_time_ns

    out = np.zeros((L, D), np.float32)
    for c in range(NCORES):
        out[1 + c * LC:1 + (c + 1) * LC] = res.results[c]["out"][0:LC]
    out[0] = res.results[0]["out"][LC]
    out[L - 1] = res.results[0]["out"][LC + 1]
    return out.reshape(1, L, D)


# revision 47
# speedup vs baseline: 1.1515x; 1.0164x over previous
"""Trainium2 Bass kernel for sparse-attention 3D-ViT (nn_BaseModel_44341242364529).

Strategy: shard the sequence axis L across 8 cores (512 patch rows each; the
BOS/EOS rows are replicated on every core as local tile 4). Per layer each
core computes its local q/k/v in bf16, AllGathers one fused bf16 k+v buffer
(DRAM collective), pulls a 1536-row causal band window of keys via
dynamic-offset DMAs (ds(pid,3) on the gathered chunk axis), and runs
band-dense attention: S^T blocked [128 keys, 8*128 queries] in PSUM (single
bf16 matmul pass per block) -> exp on ACT (scale folded) -> multiplicative
bf16 mask resident in SBUF (encodes geo-prior + validity + causal masking,
M = exp(bias)) on DVE -> P^T used as matmul stationary for AV with a
ones-column appended to V giving the softmax denominator for free.

All matmul operands are bf16 (PSUM accumulation stays fp32); the residual
stream x stays fp32 in SBUF.
"""

import numpy as np
import ml_dtypes

BF16 = ml_dtypes.bfloat16

# model dims (hardcoded per spec)
IMG, PATCH, D, H, NLAYERS, DFF = 64, 4, 256, 4, 2, 1024
GT = IMG // PATCH          # 16
N = GT * GT * GT           # 4096
L = N + 2                  # 4098
DH = D // H                # 64
PVOL = PATCH ** 3          # 64
NCORES = 8
LC = 512                   # real patch rows per core
LLOC = 640                 # padded local rows (5 tiles of 128)
NT = 5                     # local row tiles
SCALE = 1.0 / np.sqrt(DH)  # 0.125
NEG = -1e30

# per query tile t (0..3): window key-tiles [t, t+3..t+8] + local tile4 (BOS)
def _kts_for_tile(t):
    if t < 4:
        return [("win", t), ("win", t + 3), ("win", t + 4), ("win", t + 5),
                ("win", t + 6), ("win", t + 7), ("win", t + 8), ("loc4", 0)]
    # tile 4 = BOS/EOS rows: local tile4 keys + gathered global tiles 29, 31
    return [("loc4", 0), ("x", 0), ("x", 1)]


_prog_cache = {}


def _build_program(zero_flags):
    import concourse.bass as bass
    import concourse.bacc as bacc
    import concourse.tile as tile
    from concourse import mybir

    f32 = mybir.dt.float32
    bf16 = mybir.dt.bfloat16
    AF = mybir.ActivationFunctionType
    nc = bacc.Bacc("TRN2", target_bir_lowering=False, debug=False,
                   num_devices=NCORES)

    # ---------------- I/O declarations ----------------
    def din(name, shape, dt=bf16):
        return nc.declare_dram_parameter(name, list(shape), dt, isOutput=False)

    imgT_d = din("imgT", [PVOL, LLOC])
    emb_d = din("emb", [LLOC, D], f32)
    ident_d = din("ident", [128, 128])
    wq_d = din("wq", [NLAYERS, D, D])
    wk_d = din("wk", [NLAYERS, D, D])
    wv_d = din("wv", [NLAYERS, D, D])
    wo_d = din("wo", [NLAYERS, D, D])
    w1_d = din("w1", [NLAYERS, D, DFF])
    w2_d = din("w2", [NLAYERS, DFF, D])
    pw_d = din("patch_w", [PVOL, D])
    maskA_d = din("maskA", [4, H, 128, 8 * 128])     # query tiles 0..3
    maskB_d = din("maskB", [H, 128, 3 * 128])        # query tile 4
    out_d = nc.declare_dram_parameter("out", [LLOC, D], f32, isOutput=True)

    # internal DRAM for the fused k+v collective (bf16)
    # layout cols: [0:1024] = kT (dt,512), [1024:2048] = v (t,h,dh)
    kv_cc = nc.dram_tensor("kv_cc", [128, 2048], bf16)
    kv_gat = nc.dram_tensor("kv_gat", [NCORES + 2, 128, 2048], bf16,
                            addr_space="Shared")

    from contextlib import ExitStack
    with tile.TileContext(nc) as tc, ExitStack() as ctx:
        sing = ctx.enter_context(tc.tile_pool(name="sing", bufs=1))
        wk_pool = ctx.enter_context(tc.tile_pool(name="wrk", bufs=1))
        wk2_pool = ctx.enter_context(tc.tile_pool(name="wrk2", bufs=2))
        p_pool = ctx.enter_context(tc.tile_pool(name="pp", bufs=3))
        ps_big = ctx.enter_context(tc.tile_pool(name="psb", bufs=2, space="PSUM"))
        ps_sm = ctx.enter_context(tc.tile_pool(name="pss", bufs=2, space="PSUM"))
        ps_tr = ctx.enter_context(tc.tile_pool(name="pst", bufs=2, space="PSUM"))

        sync = nc.sync
        pid = sync.partition_id()

        # ---------------- load constants/weights ----------------
        ident = sing.tile([128, 128], bf16, tag="ident")
        sync.dma_start(out=ident[:], in_=ident_d[:, :])
        imgT = sing.tile([PVOL, LLOC], bf16, tag="imgT")
        sync.dma_start(out=imgT[:], in_=imgT_d[:, :])
        emb = sing.tile([128, NT, D], f32, tag="emb")
        sync.dma_start(out=emb[:], in_=emb_d.rearrange("(t p) d -> p t d", p=128))
        pw = sing.tile([PVOL, D], bf16, tag="pw")
        sync.dma_start(out=pw[:], in_=pw_d[:, :])

        W = {}
        for nm, dt_, kd in (("wq", wq_d, 2), ("wk", wk_d, 2), ("wv", wv_d, 2),
                            ("wo", wo_d, 2), ("w1", w1_d, 2), ("w2", w2_d, 8)):
            nout = dt_.shape[2]
            for l in range(NLAYERS):
                t_ = sing.tile([128, kd, nout], bf16, tag=f"{nm}{l}")
                sync.dma_start(out=t_[:], in_=dt_[l].rearrange("(k p) n -> p k n", p=128))
                W[(nm, l)] = t_

        # resident multiplicative masks (bf16)
        mA = sing.tile([128, 4, H, 8 * 128], bf16, tag="mA")
        sync.dma_start(out=mA[:], in_=maskA_d.rearrange("t h p x -> p t h x"))
        mB = sing.tile([128, H, 3 * 128], bf16, tag="mB")
        sync.dma_start(out=mB[:], in_=maskB_d.rearrange("h p x -> p h x"))

        # zero the 2 pad chunks of the gathered buffer (avoid NaN garbage)
        zt = sing.tile([128, 2048], bf16, tag="zero")
        nc.vector.memset(zt[:], 0.0)
        for ch in range(2):
            sync.dma_start(out=kv_gat[ch], in_=zt[:])

        eps_sb = sing.tile([128, 1], f32, tag="eps")
        nc.vector.memset(eps_sb[:], 1e-5)

        # persistent activations
        x_sb = wk_pool.tile([128, NT, D], f32, tag="x")
        # window k^T, one tile per feature-half so the S matmuls of heads 0/1
        # only gate on the dt=0 window DMA: [p(dh), chunk, 512]
        kT_win0 = wk_pool.tile([128, 3, 512], bf16, tag="kwin0")
        kT_win1 = wk_pool.tile([128, 3, 512], bf16, tag="kwin1")
        kT_wins = (kT_win0, kT_win1)
        # window v: [p(key row), chunk, lt, head, dh+1] (ones col written once)
        v_win = wk_pool.tile([128, 3, 4, H, DH + 1], bf16, tag="vwin")
        kT_x = wk_pool.tile([128, 2, 2, 128], bf16, tag="kx")
        v_x = wk_pool.tile([128, 2, H, DH + 1], bf16, tag="vx")
        v_ext = wk_pool.tile([128, NT, H, DH + 1], bf16, tag="vext")
        nc.vector.memset(v_win[:, :, :, :, DH:DH + 1], 1.0)
        nc.vector.memset(v_x[:, :, :, DH:DH + 1], 1.0)
        nc.vector.memset(v_ext[:, :, :, DH:DH + 1], 1.0)

        # ---------------- patch embed ----------------
        for lt in range(NT):
            ps = ps_sm.tile([128, 260], f32, tag="sm")
            nc.tensor.matmul(ps[:, 0:D], lhsT=imgT[:, lt * 128:(lt + 1) * 128],
                             rhs=pw[:], start=True, stop=True)
            nc.vector.tensor_add(x_sb[:, lt, :], ps[:, 0:D], emb[:, lt, :])

        # ---------------- helpers ----------------
        def layer_norm(src, dst, sname):
            """row-wise LN over D; scale/bias skipped when trivially 1/0."""
            for lt in range(NT):
                stats = wk2_pool.tile([128, 6], f32, tag="bns")
                mv = wk2_pool.tile([128, 2], f32, tag="bnm")
                nc.vector.bn_stats(out=stats[:], in_=src[:, lt, :])
                nc.vector.bn_aggr(out=mv[:], in_=stats[:])
                rstd = wk2_pool.tile([128, 1], f32, tag="rstd")
                nc.scalar.activation(out=rstd[:], in_=mv[:, 1:2], func=AF.Sqrt,
                                     bias=eps_sb[:], scale=1.0)
                nc.vector.reciprocal(out=rstd[:], in_=rstd[:])
                nc.vector.tensor_scalar(out=dst[:, lt, :], in0=src[:, lt, :],
                                        scalar1=mv[:, 0:1], scalar2=rstd[:],
                                        op0=mybir.AluOpType.subtract,
                                        op1=mybir.AluOpType.mult)
                if not zero_flags[sname]:
                    sc = W[("lns", sname)]
                    nc.vector.tensor_mul(dst[:, lt, :], dst[:, lt, :], sc[:, 0, :])
                    nc.vector.tensor_add(dst[:, lt, :], dst[:, lt, :], sc[:, 1, :])

        def transpose_tiles(src_sb, lt, dst_sb):
            """src [128l, 256] tile lt -> dst^T [128, 2, *] cols lt*128.."""
            for dt_ in range(2):
                pt = ps_tr.tile([128, 128], bf16, tag="tr")
                nc.tensor.transpose(pt[:], src_sb[:, lt, dt_ * 128:(dt_ + 1) * 128],
                                    ident[:])
                nc.vector.tensor_copy(out=dst_sb[:, dt_, lt * 128:(lt + 1) * 128],
                                      in_=pt[:])

        # LN scale/bias tiles if needed
        for nm in ("ln1_0", "ln2_0", "ln1_1", "ln2_1", "lnf"):
            if not zero_flags[nm]:
                t_ = sing.tile([128, 2, D], f32, tag=f"lns_{nm}")
                W[("lns", nm)] = t_
                dd = nc.declare_dram_parameter(f"lnsb_{nm}", [2, D], f32, isOutput=False)
                sync.dma_start(out=t_[:], in_=dd.to_broadcast([128, 2, D]))

        h_sb = wk_pool.tile([128, NT, D], bf16, tag="h")
        hf_sb = wk_pool.tile([128, NT, D], f32, tag="hf")
        hT = wk_pool.tile([128, 2, LLOC], bf16, tag="hT")
        qT = wk_pool.tile([128, 2, LLOC], bf16, tag="qT")
        kT = wk_pool.tile([128, 2, LLOC], bf16, tag="kT")
        yT_sb = wk_pool.tile([128, 8, LLOC], bf16, tag="yT")

        # ---------------- layers ----------------
        for l in range(NLAYERS):
            layer_norm(x_sb, h_sb, f"ln1_{l}")
            for lt in range(NT):
                transpose_tiles(h_sb, lt, hT)

            # k^T feature-major [128, 2, 640] (first: feeds the collective)
            def projT(nm, dstT):
                wsb = W[(nm, l)]
                for j in range(2):
                    ps = ps_big.tile([128, 1024], f32, tag="big")
                    for i in range(2):
                        nc.tensor.matmul(ps[:, 0:512],
                                         lhsT=wsb[:, i, j * 128:(j + 1) * 128],
                                         rhs=hT[:, i, 0:512],
                                         start=(i == 0), stop=(i == 1))
                        nc.tensor.matmul(ps[:, 512:640],
                                         lhsT=wsb[:, i, j * 128:(j + 1) * 128],
                                         rhs=hT[:, i, 512:640],
                                         start=(i == 0), stop=(i == 1))
                    nc.vector.tensor_copy(out=dstT[:, j, :], in_=ps[:, 0:LLOC])

            projT("wk", kT)

            # v row-major [128, 5, H, 65] (ones column persistent)
            wsb = W[("wv", l)]
            for lt in range(NT):
                ps = ps_sm.tile([128, 260], f32, tag="sm")
                for i in range(2):
                    nc.tensor.matmul(ps[:, 0:D],
                                     lhsT=hT[:, i, lt * 128:(lt + 1) * 128],
                                     rhs=wsb[:, i, :], start=(i == 0), stop=(i == 1))
                nc.vector.tensor_copy(
                    out=v_ext[:, lt, :, 0:DH],
                    in_=ps[:, 0:D].rearrange("p (h x) -> p h x", h=H))

            # ---- fused k+v AllGather (bf16) ----
            sync.dma_start(out=kv_cc[:, 0:1024].rearrange("p (a x) -> p a x", a=2),
                           in_=kT[:, :, 0:LC])
            sync.dma_start(out=kv_cc[:, 1024:2048].rearrange(
                               "p (t h x) -> p t h x", t=4, h=H),
                           in_=v_ext[:, 0:4, :, 0:DH])
            nc.gpsimd.collective_compute(
                "AllGather", mybir.AluOpType.bypass,
                replica_groups=[list(range(NCORES))],
                ins=[kv_cc[:, :].opt()],
                outs=[kv_gat[2:NCORES + 2].opt()])

            # q^T overlaps the collective (local only)
            projT("wq", qT)

            # ---- window DMAs (dynamic chunk offset = pid) ----
            for dt_ in range(2):
                sync.dma_start(
                    out=kT_wins[dt_][:, :, :],
                    in_=kv_gat[bass.ds(pid, 3), :, dt_ * 512:(dt_ + 1) * 512]
                        .rearrange("c p x -> p c x"))
            for c_ in range(3):
                sync.dma_start(
                    out=v_win[:, c_, :, :, 0:DH],
                    in_=kv_gat[bass.ds(pid + c_, 1), :, 1024:2048]
                        .rearrange("c p (t h x) -> p (c t) h x", t=4, h=H))
            # tile-4 extra keys: global patch tiles 29 and 31 (chunk 7 -> gat 9)
            for xi, base in ((0, 128), (1, 384)):
                sync.dma_start(
                    out=kT_x[:, :, xi, :],
                    in_=kv_gat[9, :, 0:1024].rearrange("p (a x) -> p a x", x=512)
                        [:, :, base:base + 128])
            for xi, tx in ((0, 1), (1, 3)):
                sync.dma_start(
                    out=v_x[:, xi, :, 0:DH],
                    in_=kv_gat[9, :, 1024 + tx * 256:1024 + (tx + 1) * 256]
                        .rearrange("p (h x) -> p h x", h=H))

            # PE warm-up: ~4.5us of dense back-to-back matmul gated on the
            # gathered window landing. The HAM clock-gate needs a fully-busy
            # ~3.4us window to flip to 8/8 (2.4 GHz); attention's bursty duty
            # cycle can neither flip it warm nor drop it cold, so it runs the
            # whole phase at whatever clock it entered with. This makes it
            # enter warm instead of post-idle cold.
            for _ in range(40):
                wp = ps_tr.tile([128, 128], bf16, tag="tr")
                nc.tensor.transpose(wp[:], kT_wins[0][:, 0, 0:128], ident[:])

            # ---- attention per query tile / head ----
            for t in range(NT):
                kts = _kts_for_tile(t)
                nkt = len(kts)
                ao_ps = ps_sm.tile([128, 260], f32, tag="sm")
                for hh in range(H):
                    pb, dt_ = (hh % 2) * 64, hh // 2
                    st = ps_big.tile([128, 1024], f32, tag="big")
                    for ki, (kind, w) in enumerate(kts):
                        if kind == "win":
                            lhsT = kT_wins[dt_][pb:pb + 64, w // 4,
                                               (w % 4) * 128:(w % 4 + 1) * 128]
                        elif kind == "loc4":
                            lhsT = kT[pb:pb + 64, dt_, 512:640]
                        else:
                            lhsT = kT_x[pb:pb + 64, dt_, w, :]
                        nc.tensor.matmul(st[:, ki * 128:(ki + 1) * 128], lhsT=lhsT,
                                         rhs=qT[pb:pb + 64, dt_, t * 128:(t + 1) * 128],
                                         start=True, stop=True)
                    # exp (scale folded), then multiplicative mask (bf16, DVE 2x)
                    pe = p_pool.tile([128, 1024], bf16, tag="pe")
                    nc.scalar.activation(out=pe[:, 0:nkt * 128],
                                         in_=st[:, 0:nkt * 128],
                                         func=AF.Exp, scale=float(SCALE))
                    pt_sb = p_pool.tile([128, 1024], bf16, tag="pt")
                    if t < 4:
                        msk = mA[:, t, hh, 0:nkt * 128]
                    else:
                        msk = mB[:, hh, :]
                    nc.vector.tensor_mul(pt_sb[:, 0:nkt * 128],
                                         pe[:, 0:nkt * 128], msk)
                    for ki, (kind, w) in enumerate(kts):
                        if kind == "win":
                            rhs = v_win[:, w // 4, w % 4, hh, :]
                        elif kind == "loc4":
                            rhs = v_ext[:, 4, hh, :]
                        else:
                            rhs = v_x[:, w, hh, :]
                        nc.tensor.matmul(ao_ps[:, hh * 65:hh * 65 + 65],
                                         lhsT=pt_sb[:, ki * 128:(ki + 1) * 128],
                                         rhs=rhs, start=(ki == 0), stop=(ki == nkt - 1))
                # normalize: divide by denom col, pack to row-major [128, 256]
                rec = wk2_pool.tile([128, 4], f32, tag="rec")
                nc.vector.reciprocal(out=rec[:], in_=ao_ps[:, 64:260:65])
                ao_sb = wk2_pool.tile([128, D], bf16, tag="ao")
                for hh in range(H):
                    nc.vector.tensor_scalar(
                        out=ao_sb[:, hh * DH:(hh + 1) * DH],
                        in0=ao_ps[:, hh * 65:hh * 65 + DH],
                        scalar1=rec[:, hh:hh + 1], scalar2=None,
                        op0=mybir.AluOpType.mult)
                # wo projection + residual
                aoT = wk2_pool.tile([128, 2, 128], bf16, tag="aoT")
                for dt_ in range(2):
                    ptr = ps_tr.tile([128, 128], bf16, tag="tr")
                    nc.tensor.transpose(ptr[:], ao_sb[:, dt_ * 128:(dt_ + 1) * 128],
                                        ident[:])
                    nc.vector.tensor_copy(out=aoT[:, dt_, :], in_=ptr[:])
                xo = ps_sm.tile([128, 260], f32, tag="sm")
                wsb = W[("wo", l)]
                for i in range(2):
                    nc.tensor.matmul(xo[:, 0:D], lhsT=aoT[:, i, :], rhs=wsb[:, i, :],
                                     start=(i == 0), stop=(i == 1))
                nc.vector.tensor_add(x_sb[:, t, :], x_sb[:, t, :], xo[:, 0:D])

            # ---- FFN ----
            layer_norm(x_sb, h_sb, f"ln2_{l}")
            for lt in range(NT):
                transpose_tiles(h_sb, lt, hT)
            w1sb = W[("w1", l)]
            for fj in range(8):
                ps = ps_big.tile([128, 1024], f32, tag="big")
                for i in range(2):
                    nc.tensor.matmul(ps[:, 0:512],
                                     lhsT=w1sb[:, i, fj * 128:(fj + 1) * 128],
                                     rhs=hT[:, i, 0:512], start=(i == 0), stop=(i == 1))
                    nc.tensor.matmul(ps[:, 512:640],
                                     lhsT=w1sb[:, i, fj * 128:(fj + 1) * 128],
                                     rhs=hT[:, i, 512:640], start=(i == 0), stop=(i == 1))
                nc.scalar.activation(out=yT_sb[:, fj, :], in_=ps[:, 0:LLOC],
                                     func=AF.Gelu, scale=1.0)
            w2sb = W[("w2", l)]
            for lt in range(NT):
                ps = ps_sm.tile([128, 260], f32, tag="sm")
                for fj in range(8):
                    nc.tensor.matmul(ps[:, 0:D],
                                     lhsT=yT_sb[:, fj, lt * 128:(lt + 1) * 128],
                                     rhs=w2sb[:, fj, :], start=(fj == 0), stop=(fj == 7))
                nc.vector.tensor_add(x_sb[:, lt, :], x_sb[:, lt, :], ps[:, 0:D])

        # ---------------- final LN + output ----------------
        layer_norm(x_sb, hf_sb, "lnf")
        for lt in range(NT):
            sync.dma_start(out=out_d[lt * 128:(lt + 1) * 128, :], in_=hf_sb[:, lt, :])

    nc.finalize()
    return nc


# ======================= host side =======================

def _patchify(img):
    x = img.reshape(1, 1, GT, PATCH, GT, PATCH, GT, PATCH)
    x = np.einsum("nctphqwr->nthwpqrc", x).reshape(N, PVOL)
    return np.ascontiguousarray(x).astype(np.float32)


def _host_prep(inputs):
    idx = np.asarray(inputs["idx"])
    valid = np.asarray(inputs["valid"])
    geo = np.asarray(inputs["geo_dist"]).astype(np.float32)
    decay = np.asarray(inputs["decay"]).astype(np.float32)
    K = idx.shape[1]
    fv = valid & (idx <= np.arange(L)[:, None])
    # device computes exp(SCALE*S) * M with M = exp(geo*decay) (0 if masked)
    bias_lk = geo[None] * decay[:, None, None]  # [H, L, K]

    patches = _patchify(np.asarray(inputs["input_image"]))
    ids = np.asarray(inputs["input_ids"]).reshape(-1)
    et = np.asarray(inputs["embed_tokens"])
    pb = np.asarray(inputs["patch_b"]).astype(np.float32)
    bos_e, eos_e = et[ids[0]], et[ids[-1]]

    per_core = []
    for c in range(NCORES):
        imgT = np.zeros((PVOL, LLOC), np.float32)
        imgT[:, 0:LC] = patches[c * LC:(c + 1) * LC].T
        emb = np.zeros((LLOC, D), np.float32)
        emb[0:LC] = pb[None, :]
        emb[LC] = bos_e
        emb[LC + 1] = eos_e

        biasA = np.full((4, H, 128, 8 * 128), NEG, np.float32)
        biasB = np.full((H, 128, 3 * 128), NEG, np.float32)
        base = c * LC - 1024   # window global patch start
        for lq in range(LC):
            gq = 1 + c * LC + lq
            t, lcol = lq // 128, lq % 128
            kts = [t, t + 3, t + 4, t + 5, t + 6, t + 7, t + 8]
            for k in range(K):
                if not fv[gq, k]:
                    continue
                kr = int(idx[gq, k])
                bv = bias_lk[:, gq, k]
                if kr == 0:                      # BOS -> local tile4 slot, j=0
                    biasA[t, :, 0, 7 * 128 + lcol] = bv
                    continue
                p = kr - 1
                wp = p - base
                assert 0 <= wp < 1536, (c, gq, kr)
                w, j = wp // 128, wp % 128
                ki = kts.index(w)
                biasA[t, :, j, ki * 128 + lcol] = bv
        # padding queries (tile4 rows 2..127) attend BOS only -> finite output
        biasB[:, 0, 0 * 128 + 2:0 * 128 + 128] = 0.0
        # tile 4: BOS (l=0) and EOS (l=1) queries
        for li, gq in ((0, 0), (1, L - 1)):
            for k in range(K):
                if not fv[gq, k]:
                    continue
                kr = int(idx[gq, k])
                bv = bias_lk[:, gq, k]
                if kr == 0:
                    biasB[:, 0, 0 * 128 + li] = bv
                elif kr == L - 1:
                    biasB[:, 1, 0 * 128 + li] = bv
                else:
                    p = kr - 1
                    if 3712 <= p < 3840:
                        biasB[:, p - 3712, 1 * 128 + li] = bv
                    elif 3968 <= p < 4096:
                        biasB[:, p - 3968, 2 * 128 + li] = bv
                    else:
                        raise AssertionError((gq, kr))
        per_core.append({
            "imgT": imgT.astype(BF16), "emb": emb,
            "maskA": np.exp(biasA).astype(BF16),
            "maskB": np.exp(biasB).astype(BF16),
        })

    shared = {
        "ident": np.eye(128, dtype=np.float32).astype(BF16),
        "patch_w": np.asarray(inputs["patch_w"], np.float32).astype(BF16),
    }
    for nm in ("wq", "wk", "wv", "wo", "w1", "w2"):
        shared[nm] = np.asarray(inputs[nm], np.float32).astype(BF16)

    zero_flags = {}
    for nm, s_, b_ in (("ln1_0", inputs["ln1_s"][0], inputs["ln1_b"][0]),
                       ("ln2_0", inputs["ln2_s"][0], inputs["ln2_b"][0]),
                       ("ln1_1", inputs["ln1_s"][1], inputs["ln1_b"][1]),
                       ("ln2_1", inputs["ln2_s"][1], inputs["ln2_b"][1]),
                       ("lnf", inputs["norm_s"], inputs["norm_b"])):
        s_, b_ = np.asarray(s_), np.asarray(b_)
        triv = bool(np.all(s_ == 1.0) and np.all(b_ == 0.0))
        zero_flags[nm] = triv
        if not triv:
            shared[f"lnsb_{nm}"] = np.stack([s_, b_]).astype(np.float32)
    # residual biases: asserted zero (true for this model's setup_inputs)
    for nm in ("bo", "b1", "b2"):
        assert np.all(np.asarray(inputs[nm]) == 0.0), f"{nm} nonzero unsupported"

    return per_core, shared, zero_flags


def kernel(**inputs):
    from concourse.bass_utils import run_bass_kernel_spmd

    per_core, shared, zero_flags = _host_prep(inputs)
    key = tuple(sorted(zero_flags.items()))
    if key not in _prog_cache:
        _prog_cache[key] = _build_program(zero_flags)
    nc = _prog_cache[key]

    in_maps = []
    for c in range(NCORES):
        m = dict(shared)
        m.update(per_core[c])
        in_maps.append(m)
    import os
    trace = bool(os.environ.get("KERNEL_TRACE"))
    res = run_bass_kernel_spmd(nc, in_maps, core_ids=list(range(NCORES)),
                               trace=trace)
    global _last_exec_ns
    _last_exec_ns = res.exec_time_ns

    out = np.zeros((L, D), np.float32)
    for c in range(NCORES):
        out[1 + c * LC:1 + (c + 1) * LC] = res.results[c]["out"][0:LC]
    out[0] = res.results[0]["out"][LC]
    out[L - 1] = res.results[0]["out"][LC + 1]
    return out.reshape(1, L, D)
